# revision 36
# baseline (speedup 1.0000x reference)
"""Trainium2 Bass kernel for AttentionFact:
    scores = einsum('bsh,ch->bcs', hidden, querys)
    factor = softmax(scores, axis=2)
    out    = einsum('bcs,bsh->bch', factor, hidden).reshape(B, C*H)

Shapes: B=16, S=4096, H=1024, C=64, fp32.

Strategy: data-parallel over batch, 2 batches per core, querys
replicated (host pre-transposes into [128, 8, 64] h-chunk layout).

Streaming (flash-style) softmax, v3 schedule:
  - per 512-row s-tile: hT via PE transpose, scores in PSUM over 8
    h-chunks, one PSUM->SBUF evac, per-tile reduce_max + exp (fp16)
    with fused f32 row-sum
  - per half-batch: beta = exp(m_tile - M_half) folded into a row
    scale of the fp16 exp tiles; half accumulators combined as
    (psA*gA + psB*gB)/S at the end
  - PE emission is software-pipelined ACROSS tiles: tile t+1's
    transpose block is emitted before tile t's scores matmuls so the
    PSUM evacuation copies are never on the PE critical path (keeps
    the PE p-state hot), and phase3 items are interleaved into the
    next half's phase1 through a pending queue so the PE never idles
    at half boundaries.
"""

import numpy as np

import concourse.bass as bass
import concourse.mybir as mybir
import concourse.tile as tile
from concourse import bacc
from concourse.bass_utils import run_bass_kernel_spmd

B, S, H, C = 16, 4096, 1024, 64
NCORES = 8
BPC = B // NCORES          # batches per core
ST = 8                     # s-tiles per batch (512 rows each)
SQ = 4                     # 128-row subtiles per s-tile
HJ = H // 128              # h-chunks (8)
LOOKAHEAD = 4              # tile loads kept in flight ahead of compute
SPLITS = {0: 4, 1: 5}      # half-A size per batch

F32 = mybir.dt.float32
F16 = mybir.dt.float16

# (b, st, jp) triples whose hT blocks go via DMA xbar transpose instead
# of the PE (jp indexes pairs of h-chunks, 0..3)
XBAR_JPS = set()


def build_nc():
    nc = bacc.Bacc("TRN2", target_bir_lowering=False, debug=False)
    hidden = nc.declare_dram_parameter("hidden", [BPC, S, H], F32, isOutput=False)
    qT = nc.declare_dram_parameter("qT", [128, HJ, C], F16, isOutput=False)
    ident = nc.declare_dram_parameter("ident", [128, 128], F16, isOutput=False)
    out = nc.declare_dram_parameter("out", [BPC, C, H], F32, isOutput=True)

    with tile.TileContext(nc) as tc:
        with (
            tc.tile_pool(name="const", bufs=1) as const_pool,
            tc.tile_pool(name="nat", bufs=2 * ST) as nat_pool,
            tc.tile_pool(name="hT", bufs=8) as hT_pool,
            tc.tile_pool(name="scp", bufs=2) as sc_pool,
            tc.tile_pool(name="expp", bufs=2) as exp_pool,
            tc.tile_pool(name="fT", bufs=2) as fT_pool,
            tc.tile_pool(name="stats", bufs=2) as stats_pool,
            tc.tile_pool(name="outp", bufs=2) as out_pool,
            tc.tile_pool(name="psT", bufs=3, space="PSUM") as psT_pool,
            tc.tile_pool(name="psS", bufs=1, space="PSUM") as psS_pool,
            tc.tile_pool(name="psR", bufs=2, space="PSUM") as psR_pool,
        ):
            ident_sb = const_pool.tile([128, 128], F16, tag="ident")
            nc.sync.dma_start(out=ident_sb[:], in_=ident[:])
            qT_sb = const_pool.tile([128, HJ, C], F16, tag="qT")
            nc.sync.dma_start(out=qT_sb[:], in_=qT[:])

            nat_tiles = {}
            hT_sets = {}
            exp_tiles = {}
            st_stats = {}   # b -> stats tile [64, 32] f32 laid out below
            ps_half = {}    # (b, half) -> psum accumulator [C, H]

            # stats tile columns: 0:8 negm (-tile max), 8:16 rowsum,
            # 16:24 beta, 24:26 negM half, 26:28 gamma half, 28:30 S halves
            NEGM, RS, BETA, NEGM_H, GAM_H, SH = 0, 8, 16, 24, 26, 28

            issued = []

            def load_tile(b, st):
                nat_t = nat_pool.tile([128, SQ, H], F16, tag="nat",
                                      name=f"nat{b}_{st}")
                nat_tiles[(b, st)] = nat_t
                if (b, st) == (0, 0):
                    # split the pipeline-fill load so the first transposes
                    # can start after ~a quarter tile
                    for q in range(SQ):
                        src = hidden[b, st * 512 + q * 128:
                                     st * 512 + (q + 1) * 128, :]
                        nc.gpsimd.dma_start(out=nat_t[:, q, :], in_=src)
                else:
                    src = hidden[b, st * 512:(st + 1) * 512, :].rearrange(
                        "(q p) h -> p q h", p=128
                    )
                    nc.gpsimd.dma_start(out=nat_t[:], in_=src)
                issued.append((b, st))

            def ensure_loads(upto):
                for gi in range(len(issued), min(upto + 1, BPC * ST)):
                    load_tile(gi // ST, gi % ST)

            def emit_Tblock(b, st):
                """Produce the 4 hT tiles (8 h-chunks) for s-tile st."""
                ensure_loads(b * ST + st + LOOKAHEAD)
                nat_t = nat_tiles[(b, st)]
                hTs = []
                for jp in range(4):
                    hT = hT_pool.tile([128, 1024], F16, tag="hT")
                    hTs.append(hT)
                    if (b, st, jp) in XBAR_JPS:
                        for ji in range(2):
                            j = jp * 2 + ji
                            for q in range(SQ):
                                nc.sync.dma_start_transpose(
                                    hT[:, ji * 512 + q * 128:
                                       ji * 512 + (q + 1) * 128],
                                    nat_t[:, q, j * 128:(j + 1) * 128],
                                )
                    else:
                        ps_t = psT_pool.tile([128, 1024], F16, tag="psT")
                        for ji in range(2):
                            j = jp * 2 + ji
                            for q in range(SQ):
                                nc.tensor.transpose(
                                    ps_t[:, ji * 512 + q * 128:
                                         ji * 512 + (q + 1) * 128],
                                    nat_t[:, q, j * 128:(j + 1) * 128],
                                    ident_sb[:],
                                )
                        if jp % 2 == 0:
                            nc.scalar.copy(hT[:], ps_t[:])
                        else:
                            nc.vector.tensor_copy(hT[:], ps_t[:])
                hT_sets[(b, st)] = hTs

            def emit_MMSM(b, st):
                """Scores matmuls for tile st + per-tile max/exp/rowsum."""
                stats = st_stats[b]
                hTs = hT_sets.pop((b, st))
                ps_sc = psS_pool.tile([C, 512], F32, tag="psS")
                for jp in range(4):
                    for ji in range(2):
                        j = jp * 2 + ji
                        nc.tensor.matmul(
                            ps_sc[:],
                            qT_sb[:, j, :],
                            hTs[jp][:, ji * 512:(ji + 1) * 512],
                            start=(j == 0),
                            stop=(j == HJ - 1),
                        )
                sc_sb = sc_pool.tile([C, 512], F32, tag="scp")
                nc.vector.tensor_copy(sc_sb[:], ps_sc[:])
                nc.vector.reduce_max(
                    stats[:, NEGM + st:NEGM + st + 1], sc_sb[:],
                    axis=mybir.AxisListType.X, negate=True,
                )
                nc.scalar.activation(
                    exp_tiles[b][:, st * 512:(st + 1) * 512],
                    sc_sb[:],
                    mybir.ActivationFunctionType.Exp,
                    bias=stats[:, NEGM + st:NEGM + st + 1],
                    accum_out=stats[:, RS + st:RS + st + 1],
                )

            def combine_half(b, half, t0, cnt):
                """negM over the half's tiles, beta per tile."""
                stats = st_stats[b]
                nc.vector.tensor_reduce(
                    stats[:, NEGM_H + half:NEGM_H + half + 1],
                    stats[:, NEGM + t0:NEGM + t0 + cnt],
                    axis=mybir.AxisListType.X, op=mybir.AluOpType.min,
                )
                # beta = exp(m_st - M_half) = exp(-negm + negM_half)
                nc.scalar.activation(
                    stats[:, BETA + t0:BETA + t0 + cnt],
                    stats[:, NEGM + t0:NEGM + t0 + cnt],
                    mybir.ActivationFunctionType.Exp,
                    bias=stats[:, NEGM_H + half:NEGM_H + half + 1],
                    scale=-1.0,
                )

            fT_sets = {}

            def phase3_T(b, st):
                """Beta-fold + factor transposes for s-tile st."""
                stats = st_stats[b]
                exp_sb = exp_tiles[b]
                # fold beta into the exp tile (row scale on the scalar engine)
                nc.scalar.mul(
                    exp_sb[:, st * 512:(st + 1) * 512],
                    exp_sb[:, st * 512:(st + 1) * 512],
                    stats[:, BETA + st:BETA + st + 1],
                )
                ps_f = psT_pool.tile([128, 1024], F16, tag="psT")
                for q in range(SQ):
                    k = st * SQ + q
                    nc.tensor.transpose(
                        ps_f[:, q * C:(q + 1) * C],
                        exp_sb[:, k * 128:(k + 1) * 128],
                        ident_sb[:C, :C],
                    )
                fT = fT_pool.tile([128, SQ * C], F16, tag="fT")
                if st % 2 == 0:
                    nc.scalar.copy(fT[:], ps_f[:, :SQ * C])
                else:
                    nc.vector.tensor_copy(fT[:], ps_f[:, :SQ * C])
                fT_sets[(b, st)] = fT

            def phase3_MM(b, st, half, first, last):
                ps_res = ps_half[(b, half)]
                fT = fT_sets.pop((b, st))
                nat_t = nat_tiles[(b, st)]
                for q in range(SQ):
                    for h2 in range(2):
                        nc.tensor.matmul(
                            ps_res[:, h2 * 512:(h2 + 1) * 512],
                            fT[:, q * C:(q + 1) * C],
                            nat_t[:, q, h2 * 512:(h2 + 1) * 512],
                            start=(first and q == 0),
                            stop=(last and q == SQ - 1),
                        )

            def finalize(b):
                """Combine the two half accumulators and write out."""
                stats = st_stats[b]
                negM_G = stats_pool.tile([C, 1], F32, tag="negMG")
                nc.vector.tensor_scalar_min(
                    negM_G[:], stats[:, NEGM_H:NEGM_H + 1],
                    stats[:, NEGM_H + 1:NEGM_H + 2],
                )
                # gamma_half = exp(M_half - M) = exp(-negM_half + negM)
                nc.scalar.activation(
                    stats[:, GAM_H:GAM_H + 2],
                    stats[:, NEGM_H:NEGM_H + 2],
                    mybir.ActivationFunctionType.Exp,
                    bias=negM_G[:],
                    scale=-1.0,
                )
                # S_half = sum_st beta_st * rowsum_st
                bs = stats_pool.tile([C, ST], F32, tag="bs")
                for st in range(ST):
                    nc.vector.tensor_scalar_mul(
                        bs[:, st:st + 1],
                        stats[:, RS + st:RS + st + 1],
                        stats[:, BETA + st:BETA + st + 1],
                    )
                nA = SPLITS[b]
                sg = stats_pool.tile([C, 4], F32, tag="sg")
                nc.vector.reduce_sum(
                    stats[:, SH:SH + 1], bs[:, 0:nA],
                    axis=mybir.AxisListType.X,
                )
                nc.vector.reduce_sum(
                    stats[:, SH + 1:SH + 2], bs[:, nA:ST],
                    axis=mybir.AxisListType.X,
                )
                nc.vector.tensor_scalar_mul(
                    sg[:, 0:1], stats[:, SH:SH + 1], stats[:, GAM_H:GAM_H + 1],
                )
                nc.vector.tensor_scalar_mul(
                    sg[:, 1:2], stats[:, SH + 1:SH + 2],
                    stats[:, GAM_H + 1:GAM_H + 2],
                )
                nc.vector.tensor_add(sg[:, 2:3], sg[:, 0:1], sg[:, 1:2])
                rinv = stats_pool.tile([C, 1], F32, tag="rinv")
                nc.vector.reciprocal(rinv[:], sg[:, 2:3])
                alph = stats_pool.tile([C, 2], F32, tag="alph")
                nc.vector.tensor_scalar_mul(
                    alph[:], stats[:, GAM_H:GAM_H + 2], rinv[:],
                )
                tmpA = out_pool.tile([C, H], F32, tag="tmpA")
                nc.vector.tensor_scalar_mul(
                    tmpA[:], ps_half[(b, 0)][:], alph[:, 0:1],
                )
                tmpB = out_pool.tile([C, H], F32, tag="tmpB")
                nc.vector.tensor_scalar_mul(
                    tmpB[:], ps_half[(b, 1)][:], alph[:, 1:2],
                )
                out_sb = out_pool.tile([C, H], F32, tag="out")
                nc.vector.tensor_add(out_sb[:], tmpA[:], tmpB[:])
                nc.sync.dma_start(out=out[b], in_=out_sb[:])

            # ---- schedule ----
            for b in range(BPC):
                exp_tiles[b] = exp_pool.tile([C, S], F16, tag="expf",
                                             name=f"expf{b}")
                st_stats[b] = stats_pool.tile([C, 32], F32, tag="stats",
                                              name=f"stats{b}")
            ensure_loads(LOOKAHEAD)

            pending = []   # deferred phase3/finalize closures

            def pop(n=1):
                for _ in range(n):
                    if pending:
                        pending.pop(0)()

            def emit_half(b, half, t0, cnt, drain_extra=False):
                tiles = list(range(t0, t0 + cnt))
                emit_Tblock(b, tiles[0])
                for i, st in enumerate(tiles):
                    if i + 1 < len(tiles):
                        emit_Tblock(b, tiles[i + 1])
                    emit_MMSM(b, st)
                    if i > 0 or half == 1:
                        pop(2 if drain_extra else 1)
                combine_half(b, half, t0, cnt)
                ps_half[(b, half)] = psR_pool.tile([C, H], F32, tag="psR",
                                                   name=f"psR{b}_{half}")
                # phase3 items pipelined one-behind: item i emits tile i's
                # transposes plus tile i-1's pooling matmuls, so the fT
                # evacuation copy is never on the PE critical path
                for i, st in enumerate(tiles):
                    def p3(b=b, st=st, half=half, i=i, tiles=tiles):
                        phase3_T(b, st)
                        if i > 0:
                            phase3_MM(b, tiles[i - 1], half,
                                      first=(i - 1 == 0), last=False)
                    pending.append(p3)

                def p3_flush(b=b, half=half, st=tiles[-1], cnt=cnt):
                    phase3_MM(b, st, half,
                              first=(cnt == 1), last=True)
                pending.append(p3_flush)

            for b in range(BPC):
                nA = SPLITS[b]
                emit_half(b, 0, 0, nA)
                emit_half(b, 1, nA, ST - nA, drain_extra=(b == BPC - 1))
                pending.append(lambda b=b: finalize(b))
            pop(len(pending))

    nc.compile()
    return nc


_NC_CACHE = None


def _get_nc():
    global _NC_CACHE
    if _NC_CACHE is None:
        _NC_CACHE = build_nc()
    return _NC_CACHE


def kernel(hidden, querys):
    hidden = np.ascontiguousarray(np.asarray(hidden), dtype=np.float32)
    querys = np.ascontiguousarray(np.asarray(querys), dtype=np.float32)
    assert hidden.shape == (B, S, H) and querys.shape == (C, H)

    # qT[k, j, c] = querys[c, j*128 + k]  (h-chunk-major transposed layout)
    qT = np.ascontiguousarray(
        querys.T.reshape(HJ, 128, C).transpose(1, 0, 2)
    ).astype(np.float16)
    ident = np.eye(128, dtype=np.float16)

    nc = _get_nc()
    in_maps = [
        {
            "hidden": np.ascontiguousarray(hidden[i * BPC:(i + 1) * BPC]),
            "qT": qT,
            "ident": ident,
        }
        for i in range(NCORES)
    ]
    res = run_bass_kernel_spmd(nc, in_maps, core_ids=list(range(NCORES)))
    global LAST_RESULTS
    LAST_RESULTS = res
    outs = [np.asarray(res.results[i]["out"]).reshape(BPC, C * H)
            for i in range(NCORES)]
    return np.concatenate(outs, axis=0)


LAST_RESULTS = None
